# revision 1
# baseline (speedup 1.0000x reference)
"""Trainium2 Bass kernel for AdaptiveDiffusionBlock (8 NeuronCores, SPMD).

Row-shards N_P=2048 over 8 cores (256 rows each). Restructured math:

    residual = Xf1@Wp0.T + Xa1@Wa0.T + Rf@(Xf1@Wp1.T) + attn1@(Xa1@Wa1.T)

Step 1 computes Xf1/Xa1 TRANSPOSED ([c, i] chunks per k) via matmuls with
x-chunks stationary and rfT / attn0T moving, so the c-contraction
projections need no device transposes of big tensors. P=Xf1@Wp1.T and
Q=Xa1@Wa1.T are all-gathered (bf16) and consumed by step-2 row-major
matmuls accumulating straight into the row-major residual. pooled1 =
attn0 @ pooled0 (linearity of the protein-axis mean), so the step-1
attention chain never needs the row-major Xa1. Top-p thresholds via
binary search on t in (0,1] (u = exp(l - rowmax), so row max == 1.0):
h(t) = sum(u * (u > t)) in one scalar_tensor_tensor+accum_out pass.

kernel(**inputs) takes full numpy inputs, returns the full output.
"""

import sys

for _p in ("/opt/trn_rl_repo", "/root/.axon_site", "/root/.axon_site/_ro/trn_rl_repo"):
    if _p not in sys.path:
        sys.path.append(_p)

import numpy as np
import ml_dtypes

from concourse import bacc, tile, mybir, masks
from concourse.bass_utils import run_bass_kernel_spmd

BF16 = mybir.dt.bfloat16
F32 = mybir.dt.float32
F8 = mybir.dt.float8e4
AX = mybir.AxisListType
OP = mybir.AluOpType
AF = mybir.ActivationFunctionType

NCORES = 8
NP_ = 2048
NC_ = 64
C_ = 128
D_ = 64
R_ = NP_ // NCORES   # 256
KC = NC_ * C_        # 8192
P_TOPP = 0.9
LN_EPS = 1e-5
N_ITER = 6
GROUPS = [list(range(NCORES))]
SCALE_STAT = 512.0   # fp8 scale on rfT / attn1T for stage 2
SCALE_PQ = 16.0      # fp8 scale on P / Q
INV_SCALE = 1.0 / (SCALE_STAT * SCALE_PQ)


def _tp128(nc, psum_tp, dst_ap, src_ap, ident, dtype, name):
    """PE transpose of a [128,128] block: src (SBUF) -> dst (SBUF)."""
    ps = psum_tp.tile([128, 128], dtype, name=name, tag="tp")
    nc.tensor.transpose(ps[:], src_ap, ident)
    nc.vector.tensor_copy(dst_ap, ps[:])


def _attention_step(nc, pools, pooledT_loc, pooled_fullT, step):
    """pooledT_loc [128c,256i], pooled_fullT [128c,2048j] (f32) ->
    two attn tiles [128, 2048] bf16 (row-major, masked + renormalized)."""
    psum_a, small = pools["psum_a"], pools["small"]
    big_lg, big_u = pools["big_lg"], pools["big_u"]
    gT_sb, w3T_sb = pools["gT_sb"], pools["w3T_sb"]

    qT_ps = psum_a.tile([64, R_], F32, name=f"qT_ps{step}", tag="attn_ps")
    nc.tensor.matmul(qT_ps[:], lhsT=gT_sb[:], rhs=pooledT_loc, start=True, stop=True)
    qT_sb = big_lg.tile([64, R_], F32, name=f"qT_sb{step}", tag="qT_sb")
    nc.scalar.copy(qT_sb[:], qT_ps[:])

    e3T_sb = big_lg.tile([64, NP_], F32, name=f"e3T_sb{step}", tag="e3T_sb")
    for n in range(4):
        e3_ps = psum_a.tile([64, 512], F32, name=f"e3_ps{step}_{n}", tag="attn_ps")
        nc.tensor.matmul(e3_ps[:], lhsT=w3T_sb[:],
                         rhs=pooled_fullT[:, n * 512:(n + 1) * 512],
                         start=True, stop=True)
        nc.scalar.copy(e3T_sb[:, n * 512:(n + 1) * 512], e3_ps[:])

    attn_tiles = []
    for mi in range(2):
        lg = big_lg.tile([128, NP_], F32, name=f"lg{step}_{mi}", tag="logits")
        for n in range(4):
            lg_ps = psum_a.tile([128, 512], F32, name=f"lg_ps{step}_{mi}_{n}",
                                tag="attn_ps")
            nc.tensor.matmul(lg_ps[:], lhsT=qT_sb[:, mi * 128:(mi + 1) * 128],
                             rhs=e3T_sb[:, n * 512:(n + 1) * 512],
                             start=True, stop=True)
            nc.scalar.copy(lg[:, n * 512:(n + 1) * 512], lg_ps[:])

        rmax = small.tile([128, 1], F32, name=f"rmax{step}_{mi}", tag="rmax")
        nc.vector.tensor_reduce(rmax[:], lg[:], axis=AX.X, op=OP.max)
        negmax = small.tile([128, 1], F32, name=f"negmax{step}_{mi}", tag="negmax")
        nc.vector.tensor_scalar_mul(negmax[:], rmax[:], -1.0)
        u = big_u.tile([128, NP_], BF16, name=f"u{step}_{mi}", tag="u")
        zp = small.tile([128, 4], F32, name=f"zp{step}_{mi}", tag="zp")
        for n in range(4):
            nc.scalar.activation(u[:, n * 512:(n + 1) * 512],
                                 lg[:, n * 512:(n + 1) * 512],
                                 AF.Exp, bias=negmax[:], scale=1.0,
                                 accum_out=zp[:, n:n + 1])
        target = small.tile([128, 1], F32, name=f"target{step}_{mi}", tag="target")
        nc.vector.tensor_reduce(target[:], zp[:], axis=AX.X, op=OP.add)
        nc.vector.tensor_scalar_mul(target[:], target[:], P_TOPP)

        t = small.tile([128, 1], F32, name=f"t{step}_{mi}", tag="t")
        t_lo = small.tile([128, 1], F32, name=f"tlo{step}_{mi}", tag="tlo")
        nc.vector.memset(t[:], 0.5)
        nc.vector.memset(t_lo[:], 0.0)
        scratch = big_u.tile([128, NP_], BF16, name=f"scr{step}_{mi}", tag="scratch")
        hsum = small.tile([128, 1], F32, name=f"h{step}_{mi}", tag="hsum")
        cond = small.tile([128, 1], F32, name=f"cond{step}_{mi}", tag="cond")
        cond_u8 = small.tile([128, 1], mybir.dt.uint8,
                             name=f"condu{step}_{mi}", tag="cond_u8")
        toff = small.tile([128, 1], F32, name=f"toff{step}_{mi}", tag="toff")
        for it in range(N_ITER):
            nc.vector.scalar_tensor_tensor(scratch[:], u[:], t[:], u[:],
                                           op0=OP.is_gt, op1=OP.mult,
                                           accum_out=hsum[:])
            nc.vector.tensor_tensor(cond[:], hsum[:], target[:], op=OP.is_ge)
            nc.vector.tensor_copy(cond_u8[:], cond[:])
            nc.vector.copy_predicated(t_lo[:], cond_u8[:], t[:])
            delta = 2.0 ** (-(it + 2))
            nc.vector.tensor_scalar_sub(toff[:], t[:], delta)
            nc.vector.scalar_tensor_tensor(t[:], cond[:], 2.0 * delta, toff[:],
                                           op0=OP.mult, op1=OP.add)

        ssum = small.tile([128, 1], F32, name=f"S{step}_{mi}", tag="ssum")
        nc.vector.scalar_tensor_tensor(scratch[:], u[:], t_lo[:], u[:],
                                       op0=OP.is_gt, op1=OP.mult,
                                       accum_out=ssum[:])
        rs = small.tile([128, 1], F32, name=f"rS{step}_{mi}", tag="rs")
        nc.vector.reciprocal(rs[:], ssum[:])
        nc.vector.tensor_scalar(u[:], scratch[:], rs[:], None, op0=OP.mult)
        attn_tiles.append(u)
    return attn_tiles


def build_graph(trivial_affine=True):
    nc = bacc.Bacc("TRN2", target_bir_lowering=False, debug=False,
                   num_devices=NCORES)

    # ---- parameters ----
    x_f8 = nc.dram_tensor("x_f8", [NP_, KC], F8, kind="ExternalInput")
    x_loc = nc.dram_tensor("x_loc", [R_, KC], F32, kind="ExternalInput")
    rfT_f8 = nc.dram_tensor("rfT_f8", [NP_, R_], F8, kind="ExternalInput")
    gT = nc.dram_tensor("gT", [C_, D_], F32, kind="ExternalInput")
    w3T = nc.dram_tensor("w3T", [C_, D_], F32, kind="ExternalInput")
    wp0T = nc.dram_tensor("wp0T", [C_, C_], F8, kind="ExternalInput")
    wp1T = nc.dram_tensor("wp1T", [C_, C_], F8, kind="ExternalInput")
    wa0T = nc.dram_tensor("wa0T", [C_, C_], F8, kind="ExternalInput")
    wa1T = nc.dram_tensor("wa1T", [C_, C_], F8, kind="ExternalInput")
    gate_col = nc.dram_tensor("gate_col", [128, 1], F32, kind="ExternalInput")
    if not trivial_affine:
        gamma_rep = nc.dram_tensor("gamma_rep", [128, C_], F32,
                                   kind="ExternalInput")
        beta_rep = nc.dram_tensor("beta_rep", [128, C_], F32,
                                  kind="ExternalInput")
    out_loc = nc.dram_tensor("out_loc", [R_, KC], F32, kind="ExternalOutput")

    with tile.TileContext(nc) as tc:
        # ---- DRAM bounce buffers (pool tiles so Tile tracks deps) ----
        dram = tc.alloc_tile_pool(name="dram", bufs=1, space="DRAM")
        warm_bounce = dram.tile([128, 1], F32, name="warm_bounce")
        warm_full = dram.tile([128 * NCORES, 1], F32, name="warm_full",
                              addr_space="Shared")
        pooled0_bounce = dram.tile([R_, C_], F32, name="pooled0_bounce")
        pooled0_full = dram.tile([NP_, C_], F32, name="pooled0_full",
                                 addr_space="Shared")
        pooled1_bounce = dram.tile([R_, C_], F32, name="pooled1_bounce")
        pooled1_full = dram.tile([NP_, C_], F32, name="pooled1_full",
                                 addr_space="Shared")
        pq_bounce = [dram.tile([R_, 8192], F8, name=f"pq_bounce{ch}")
                     for ch in range(2)]
        pq_full = [dram.tile([NP_, 8192], F8, name=f"pq_full{ch}",
                             addr_space="Shared") for ch in range(2)]

        # ---- persistent SBUF ----
        const_pool = tc.alloc_tile_pool(name="const", bufs=1)
        small_pool = tc.alloc_tile_pool(name="small", bufs=2)

        nc.gpsimd.collective_compute(
            "AllGather", OP.bypass, replica_groups=GROUPS,
            ins=[warm_bounce[:, :]], outs=[warm_full[:, :]])
        ident_f32 = const_pool.tile([128, 128], F32, name="ident_f32")
        ident_bf16 = const_pool.tile([128, 128], BF16, name="ident_bf16")
        masks.make_identity(nc, ident_f32[:])
        masks.make_identity(nc, ident_bf16[:])

        gT_sb = const_pool.tile([C_, D_], F32, name="gT_sb")
        w3T_sb = const_pool.tile([C_, D_], F32, name="w3T_sb")
        nc.sync.dma_start(gT_sb[:], gT[:, :])
        nc.sync.dma_start(w3T_sb[:], w3T[:, :])
        wps = {}
        for nm, hd in (("wp0T", wp0T), ("wp1T", wp1T), ("wa0T", wa0T),
                       ("wa1T", wa1T)):
            wsb = const_pool.tile([C_, C_], F8, name=nm + "_sb")
            nc.sync.dma_start(wsb[:], hd[:, :])
            wps[nm] = wsb
        gate_sb = const_pool.tile([128, 1], F32, name="gate_sb")
        nc.sync.dma_start(gate_sb[:], gate_col[:, :])
        if not trivial_affine:
            gamma_sb = const_pool.tile([128, C_], F32, name="gamma_sb")
            beta_sb = const_pool.tile([128, C_], F32, name="beta_sb")
            nc.sync.dma_start(gamma_sb[:], gamma_rep[:, :])
            nc.sync.dma_start(beta_sb[:], beta_rep[:, :])

        # combined moving operand for step-1: per j-chunk jb cols
        # [jb*512:+256]=rfT chunk, [jb*512+256:+256]=attn0T chunk.
        # rfT chunks double as step-2 stationary (slices).
        cat01 = const_pool.tile([128, 16 * 512], F8, name="cat01")
        for jb in range(16):
            nc.sync.dma_start(cat01[:, jb * 512:jb * 512 + 256],
                              rfT_f8[jb * 128:(jb + 1) * 128, :])
        attn1T_sb = const_pool.tile([128, 16 * 256], F8, name="attn1T_sb")
        rfT8_sb = const_pool.tile([128, 16 * 256], F8, name="rfT8_sb")
        for jb in range(16):
            nc.sync.dma_start(rfT8_sb[:, jb * 256:(jb + 1) * 256],
                              rfT_f8[jb * 128:(jb + 1) * 128, :])

        def rfT_chunk(jb):
            return cat01[:, jb * 512:jb * 512 + 256]

        def attn0T_chunk(jb):
            return cat01[:, jb * 512 + 256:(jb + 1) * 512]

        resid = [[const_pool.tile([128, 2048], BF16, name=f"resid{mi}_{cc}")
                  for cc in range(4)] for mi in range(2)]
        _pfT = const_pool.tile([128, NP_], F32, name="pooled_fullT")
        pooled_fullT = [_pfT, _pfT]
        pooled0T_loc = const_pool.tile([128, R_], F32, name="pooled0T_loc")
        pooled1T_loc = const_pool.tile([128, R_], F32, name="pooled1T_loc")

        # psum pools: transposes (2 banks) + attention (2 banks)
        psum_tp = tc.alloc_tile_pool(name="psum_tp", bufs=1, space="PSUM")
        psum_a = tc.alloc_tile_pool(name="psum_a", bufs=2, space="PSUM")

        big_lg = tc.alloc_tile_pool(name="big_lg", bufs=1)
        big_u = tc.alloc_tile_pool(name="big_u", bufs=2)

        pools = dict(psum_a=psum_a, small=small_pool, big_lg=big_lg,
                     big_u=big_u, gT_sb=gT_sb, w3T_sb=w3T_sb)

        # ================= Stage A: pooled0 =================
        pooled0_rows = [small_pool.tile([128, C_], F32, name=f"pooled0_r{mi}",
                                        tag=f"pooled0_r{mi}") for mi in range(2)]
        with tc.tile_pool(name="xl_pool", bufs=1) as xl_pool:
            for mi in range(2):
                xl = xl_pool.tile([128, KC], F32, name="xl", tag="xl")
                nc.sync.dma_start(xl[:], x_loc[mi * 128:(mi + 1) * 128, :])
                v = xl[:].rearrange("p (k c) -> p c k", k=NC_)
                nc.vector.tensor_reduce(pooled0_rows[mi][:], v, axis=AX.X,
                                        op=OP.add)
                _tp128(nc, psum_tp, pooled0T_loc[:, mi * 128:(mi + 1) * 128],
                       pooled0_rows[mi][:], ident_f32[:], F32, f"tp_p0_{mi}")
                nc.sync.dma_start(pooled0_bounce[mi * 128:(mi + 1) * 128, :],
                                  pooled0_rows[mi][:])

        nc.gpsimd.collective_compute(
            "AllGather", OP.bypass, replica_groups=GROUPS,
            ins=[pooled0_bounce[:, :]], outs=[pooled0_full[:, :]])

        with tc.tile_pool(name="pf_pool", bufs=3) as pf_pool:
            for jb in range(16):
                pt = pf_pool.tile([128, C_], F32, name="pf_t", tag="pf_t")
                nc.sync.dma_start(pt[:], pooled0_full[jb * 128:(jb + 1) * 128, :])
                _tp128(nc, psum_tp,
                       pooled_fullT[0][:, jb * 128:(jb + 1) * 128],
                       pt[:], ident_f32[:], F32, f"tp_pf0_{jb}")

        # ================= attention step 0 =================
        attn0 = _attention_step(nc, pools, pooled0T_loc[:], pooled_fullT[0][:], 0)
        for mi in range(2):
            for jb in range(16):
                dst = cat01[:, jb * 512 + 256 + mi * 128:
                            jb * 512 + 256 + (mi + 1) * 128]
                pool_ = psum_tp if jb % 2 == 0 else psum_a
                ps = pool_.tile([128, 128], BF16, name=f"tpa0_{mi}_{jb}",
                                tag="tp" if jb % 2 == 0 else "attn_ps")
                nc.tensor.transpose(ps[:], attn0[mi][:, jb * 128:(jb + 1) * 128],
                                    ident_bf16[:])
                nc.vector.tensor_scalar(dst, ps[:], SCALE_STAT, None, op0=OP.mult)

        # ================= Stage D: pooled1 + attention step 1 =================
        with tc.tile_pool(name="p0f_pool", bufs=3) as p0f_pool:
            ps1 = psum_a.tile([128, R_], F32, name="pooled1_ps", tag="attn_ps")
            for jb in range(16):
                p0c = p0f_pool.tile([128, C_], F32, name="p0c", tag="p0c")
                nc.scalar.dma_start(p0c[:], pooled0_full[jb * 128:(jb + 1) * 128, :])
                p0cb = p0f_pool.tile([128, C_], F8, name="p0cb", tag="p0cb")
                nc.vector.tensor_copy(p0cb[:], p0c[:])
                nc.tensor.matmul(ps1[:], lhsT=p0cb[:],
                                 rhs=attn0T_chunk(jb),
                                 start=(jb == 0), stop=(jb == 15))
            nc.vector.tensor_scalar(pooled1T_loc[:], ps1[:], 1.0 / SCALE_STAT,
                                    None, op0=OP.mult)

        pooled1_rows = [small_pool.tile([128, C_], F32, name=f"pooled1_r{mi}",
                                        tag=f"pooled1_r{mi}") for mi in range(2)]
        for mi in range(2):
            _tp128(nc, psum_tp, pooled1_rows[mi][:],
                   pooled1T_loc[:, mi * 128:(mi + 1) * 128], ident_f32[:],
                   F32, f"tp_p1_{mi}")
            nc.gpsimd.dma_start(pooled1_bounce[mi * 128:(mi + 1) * 128, :],
                                 pooled1_rows[mi][:])
        nc.gpsimd.collective_compute(
            "AllGather", OP.bypass, replica_groups=GROUPS,
            ins=[pooled1_bounce[:, :]], outs=[pooled1_full[:, :]])
        with tc.tile_pool(name="pf1_pool", bufs=3) as pf1_pool:
            for jb in range(16):
                pt = pf1_pool.tile([128, C_], F32, name="pf1_t", tag="pf1_t")
                nc.sync.dma_start(pt[:], pooled1_full[jb * 128:(jb + 1) * 128, :])
                _tp128(nc, psum_tp,
                       pooled_fullT[1][:, jb * 128:(jb + 1) * 128],
                       pt[:], ident_f32[:], F32, f"tp_pf1_{jb}")

        attn1 = _attention_step(nc, pools, pooled1T_loc[:], pooled_fullT[1][:], 1)
        for mi in range(2):
            for jb in range(16):
                dst = attn1T_sb[:, jb * 256 + mi * 128: jb * 256 + (mi + 1) * 128]
                pool_ = psum_tp if jb % 2 == 0 else psum_a
                ps = pool_.tile([128, 128], BF16, name=f"tpa1_{mi}_{jb}",
                                tag="tp" if jb % 2 == 0 else "attn_ps")
                nc.tensor.transpose(ps[:], attn1[mi][:, jb * 128:(jb + 1) * 128],
                                    ident_bf16[:])
                nc.vector.tensor_scalar(dst, ps[:], SCALE_STAT, None, op0=OP.mult)

        # ========== Stage B+C: step-1 transposed diffusion + projections ==========
        # XfXaT (fp8, x32) per k: cols [k*512:+256]=Xf1T[k]; [+256:+512]=Xa1T[k]
        xfxa_pool = tc.alloc_tile_pool(name="xfxa", bufs=1)
        XfXaT = xfxa_pool.tile([128, NC_ * 512], F8, name="XfXaT")
        psum_b = tc.alloc_tile_pool(name="psum_b", bufs=2, space="PSUM")
        psum_c = tc.alloc_tile_pool(name="psum_c", bufs=1, space="PSUM")
        xs_pool = tc.alloc_tile_pool(name="xstream", bufs=5)
        pq_pool = tc.alloc_tile_pool(name="pq_stage", bufs=3)
        cat_v = cat01[:].rearrange("p (jbp s w) -> p jbp s w", s=2, w=512)
        for kq in range(16):          # groups of 4 k-slices
            xts = []
            for jh in range(2):       # j-chunk halves (8 chunks each)
                xt = xs_pool.tile([128, 8, 512], F8,
                                  name=f"xt{kq}_{jh}", tag="xt")
                src_ = x_f8[jh * 1024:(jh + 1) * 1024,
                            kq * 512:(kq + 1) * 512].rearrange(
                                "(jb p) c -> p jb c", p=128)
                nc.sync.dma_start(xt[:, :4, :], src_[:, :4, :])
                nc.sync.dma_start(xt[:, 4:, :], src_[:, 4:, :])
                xts.append(xt)
            for k4 in range(4):
                k = kq * 4 + k4
                ps = psum_b.tile([128, 512], F32, name=f"psB{k}", tag="psB")
                for jbp in range(8):  # pairs of j-chunks (DoubleRow)
                    nc.tensor.matmul(
                        ps[:],
                        lhsT=xts[jbp // 4][:, (jbp % 4) * 2:(jbp % 4) * 2 + 2,
                                           k4 * 128:(k4 + 1) * 128],
                        rhs=cat_v[:, jbp, :, :],
                        start=(jbp == 0), stop=(jbp == 7),
                        perf_mode=mybir.MatmulPerfMode.DoubleRow)
                if k % 2 == 0:
                    nc.vector.tensor_scalar(XfXaT[:, k * 512:(k + 1) * 512],
                                            ps[:], 1.0 / 16.0, None, op0=OP.mult)
                else:
                    nc.scalar.mul(XfXaT[:, k * 512:(k + 1) * 512], ps[:],
                                  1.0 / 16.0)
            # projections for this 4-k group (P/Q carry SCALE_PQ; psum is x256)
            kg = kq
            ch = kg // 8
            for ig in range(2):
                ps_p = psum_c.tile([128, 512], F32, name=f"psP{ig}_{kg}",
                                   tag="psP")
                ps_q = psum_c.tile([128, 512], F32, name=f"psQ{ig}_{kg}",
                                   tag="psQ")
                ps_r = psum_c.tile([128, 512], F32, name=f"psR{ig}_{kg}",
                                   tag="psR")
                for k4 in range(4):
                    k = kg * 4 + k4
                    xfc = XfXaT[:, k * 512 + ig * 128: k * 512 + (ig + 1) * 128]
                    xac = XfXaT[:, k * 512 + 256 + ig * 128:
                                k * 512 + 256 + (ig + 1) * 128]
                    cs = slice(k4 * 128, (k4 + 1) * 128)
                    nc.tensor.matmul(ps_p[:, cs], lhsT=xfc, rhs=wps["wp1T"][:],
                                     start=True, stop=True)
                    nc.tensor.matmul(ps_r[:, cs], lhsT=xfc, rhs=wps["wp0T"][:],
                                     start=True, stop=False)
                    nc.tensor.matmul(ps_r[:, cs], lhsT=xac, rhs=wps["wa0T"][:],
                                     start=False, stop=True)
                    nc.tensor.matmul(ps_q[:, cs], lhsT=xac, rhs=wps["wa1T"][:],
                                     start=True, stop=True)
                nc.scalar.mul(
                    resid[ig][kg // 4][:, (kg % 4) * 512:(kg % 4 + 1) * 512],
                    ps_r[:], 1.0 / 256.0)
                pt = pq_pool.tile([128, 512], F8, name="p_st", tag="p_st")
                nc.scalar.mul(pt[:], ps_p[:], SCALE_PQ / 256.0)
                nc.gpsimd.dma_start(
                    pq_bounce[ch][ig * 128:(ig + 1) * 128,
                                  (kg % 8) * 512:(kg % 8 + 1) * 512], pt[:])
                qt = pq_pool.tile([128, 512], F8, name="q_st", tag="q_st")
                nc.scalar.mul(qt[:], ps_q[:], SCALE_PQ / 256.0)
                nc.gpsimd.dma_start(
                    pq_bounce[ch][ig * 128:(ig + 1) * 128,
                                  4096 + (kg % 8) * 512:
                                  4096 + (kg % 8 + 1) * 512], qt[:])
            if kg % 8 == 7:
                nc.gpsimd.collective_compute(
                    "AllGather", OP.bypass, replica_groups=GROUPS,
                    ins=[pq_bounce[ch][:, :]], outs=[pq_full[ch][:, :]])
        pq_pool.release()
        xs_pool.release()
        psum_c.release()
        psum_b.release()
        xfxa_pool.release()

        # close attention pools before stage E (frees PSUM + SBUF)
        big_u.release()
        big_lg.release()
        psum_a.release()
        psum_tp.release()

        # ================= Stage E: step-2 row-major diffusion =================
        psum_e = tc.alloc_tile_pool(name="psum_e", bufs=2, space="PSUM")
        ln_pool = tc.alloc_tile_pool(name="ln_pool", bufs=2)
        with tc.tile_pool(name="s2rhs", bufs=4) as s2_pool:
            for n in range(16):
                ch, nin = n // 8, n % 8
                pss = [psum_e.tile([128, 512], F32, name=f"psE{n}_{mi}",
                                   tag=f"psE{mi}") for mi in range(2)]
                rts = []
                for pi in range(2):
                    rt = s2_pool.tile([128, 16, 512], F8, name=f"rt{n}_{pi}",
                                      tag=f"rt{pi}")
                    srcv = pq_full[ch][:, pi * 4096 + nin * 512:
                                       pi * 4096 + (nin + 1) * 512].rearrange(
                        "(jb p) c -> p jb c", p=128)
                    for q4 in range(4):
                        nc.gpsimd.dma_start(rt[:, q4 * 4:(q4 + 1) * 4, :],
                                            srcv[:, q4 * 4:(q4 + 1) * 4, :])
                    rts.append(rt)
                rf8_v = rfT8_sb[:].rearrange("p (jb s i) -> p jb s i",
                                             s=2, i=256)
                at1_v = attn1T_sb[:].rearrange("p (jb s i) -> p jb s i",
                                               s=2, i=256)
                for mi in range(2):
                    for pi in range(2):
                        for jb in range(8):
                            lh3 = (rf8_v if pi == 0 else at1_v)[
                                :, jb, :, mi * 128:(mi + 1) * 128]
                            nc.tensor.matmul(
                                pss[mi][:], lhsT=lh3,
                                rhs=rts[pi][:, 2 * jb:2 * jb + 2, :],
                                start=(pi == 0 and jb == 0),
                                stop=(pi == 1 and jb == 7),
                                perf_mode=mybir.MatmulPerfMode.DoubleRow)
                    rsl = resid[mi][n // 4][:, (n % 4) * 512:(n % 4 + 1) * 512]
                    nc.vector.scalar_tensor_tensor(
                        rsl, pss[mi][:], INV_SCALE, rsl,
                        op0=OP.mult, op1=OP.add)
        # ================= Stage F: layernorm + output (chunked) =================
        NCH = 8
        CW = KC // NCH
        KW = NC_ // NCH           # 16 k-groups per chunk
        if True:
            for mi in range(2):
                for cc in range(NCH):
                    cs = slice(cc * CW, (cc + 1) * CW)
                    xl2 = ln_pool.tile([128, CW], F32, name=f"xl2_{mi}_{cc}",
                                       tag="xl2")
                    h = ln_pool.tile([128, CW], F32, name=f"hln_{mi}_{cc}",
                                     tag="hln")
                    nc.sync.dma_start(xl2[:], x_loc[mi * 128:(mi + 1) * 128, cs])
                    rsl = resid[mi][cc // 2][:, (cc % 2) * CW:(cc % 2 + 1) * CW]
                    nc.vector.scalar_tensor_tensor(h[:], rsl,
                                                   gate_sb[:], xl2[:],
                                                   op0=OP.mult, op1=OP.add)
                    hv = h[:].rearrange("p (k c) -> p k c", k=KW)
                    hsq = ln_pool.tile([128, CW], F32, name=f"hsq_{mi}_{cc}",
                                       tag="hsq")
                    nc.scalar.activation(hsq[:], h[:], AF.Square)
                    hsqv = hsq[:].rearrange("p (k c) -> p k c", k=KW)
                    s1 = ln_pool.tile([128, KW, 1], F32, name=f"s1_{mi}_{cc}",
                                      tag="s1")
                    s2 = ln_pool.tile([128, KW, 1], F32, name=f"s2_{mi}_{cc}",
                                      tag="s2")
                    nc.vector.tensor_reduce(s1[:], hv, axis=AX.X, op=OP.add)
                    nc.vector.tensor_reduce(s2[:], hsqv, axis=AX.X, op=OP.add)
                    mu = ln_pool.tile([128, KW, 1], F32, name=f"mu_{mi}_{cc}",
                                      tag="mu")
                    msq = ln_pool.tile([128, KW, 1], F32, name=f"msq_{mi}_{cc}",
                                       tag="msq")
                    var = ln_pool.tile([128, KW, 1], F32, name=f"var_{mi}_{cc}",
                                       tag="var")
                    sd = ln_pool.tile([128, KW, 1], F32, name=f"sd_{mi}_{cc}",
                                      tag="sd")
                    rstd = ln_pool.tile([128, KW, 1], F32, name=f"rstd_{mi}_{cc}",
                                        tag="rstd")
                    mb = ln_pool.tile([128, KW, 1], F32, name=f"mb_{mi}_{cc}",
                                      tag="mb")
                    nc.vector.tensor_scalar_mul(mu[:], s1[:], 1.0 / C_)
                    nc.vector.tensor_tensor(msq[:], mu[:], mu[:], op=OP.mult)
                    nc.vector.tensor_scalar(var[:], s2[:], 1.0 / C_, LN_EPS,
                                            op0=OP.mult, op1=OP.add)
                    nc.vector.tensor_tensor(var[:], var[:], msq[:],
                                            op=OP.subtract)
                    nc.scalar.activation(sd[:], var[:], AF.Sqrt)
                    nc.vector.reciprocal(rstd[:], sd[:])
                    nc.vector.tensor_tensor(mb[:], mu[:], rstd[:], op=OP.mult)
                    nc.vector.tensor_scalar_mul(mb[:], mb[:], -1.0)
                    ov = xl2[:].rearrange("p (k c) -> p k c", k=KW)
                    rstd_bc = rstd[:].broadcast_to([128, KW, C_])
                    mb_bc = mb[:].broadcast_to([128, KW, C_])
                    nc.vector.tensor_tensor(ov, hv, rstd_bc, op=OP.mult)
                    nc.vector.tensor_tensor(ov, ov, mb_bc, op=OP.add)
                    if not trivial_affine:
                        g_bc = gamma_sb[:].rearrange(
                            "p (one c) -> p one c", one=1).broadcast_to(
                                [128, KW, C_])
                        b_bc = beta_sb[:].rearrange(
                            "p (one c) -> p one c", one=1).broadcast_to(
                                [128, KW, C_])
                        nc.vector.tensor_tensor(ov, ov, g_bc, op=OP.mult)
                        nc.vector.tensor_tensor(ov, ov, b_bc, op=OP.add)
                    nc.sync.dma_start(out_loc[mi * 128:(mi + 1) * 128, cs],
                                      xl2[:])

        ln_pool.release()
        psum_e.release()
        small_pool.release()
        const_pool.release()
        dram.release()

    nc.finalize()
    return nc


# ---------------------------------------------------------------------------
# Host side
# ---------------------------------------------------------------------------
_CACHE = {}


def _get_graph(trivial_affine):
    key = bool(trivial_affine)
    if key not in _CACHE:
        _CACHE[key] = build_graph(key)
    return _CACHE[key]


def prepare_in_maps(x, prior, W1, W2, W3, prior_fwd_w, adaptive_w,
                    ln_gamma, ln_beta, alpha):
    bf = ml_dtypes.bfloat16
    x2 = np.ascontiguousarray(np.asarray(x, np.float32).reshape(NP_, KC))
    x_f8 = x2.astype(ml_dtypes.float8_e4m3)
    prior = np.asarray(prior, np.float32)
    rs = np.maximum(prior.sum(axis=1, keepdims=True), 1e-12)
    rf = (prior / rs).astype(np.float32)

    W1 = np.asarray(W1, np.float32)
    W2 = np.asarray(W2, np.float32)
    W3 = np.asarray(W3, np.float32)
    G = (W2 @ W1)                       # [D, C]
    gT_h = np.ascontiguousarray(G.T) / np.float32(NC_)       # [C, D]
    w3T_h = np.ascontiguousarray(W3.T) / np.float32(NC_)     # [C, D]

    pw = np.asarray(prior_fwd_w, np.float32)
    aw = np.asarray(adaptive_w, np.float32)
    f8 = ml_dtypes.float8_e4m3
    wp0T = (np.ascontiguousarray(pw[0].T) * 8.0).astype(f8)
    wp1T = (np.ascontiguousarray(pw[1].T) * 8.0).astype(f8)
    wa0T = (np.ascontiguousarray(aw[0].T) * 8.0).astype(f8)
    wa1T = (np.ascontiguousarray(aw[1].T) * 8.0).astype(f8)

    gate = 1.0 / (1.0 + np.exp(-np.float32(np.asarray(alpha).reshape(-1)[0])))
    gate_col = np.full((128, 1), gate, np.float32)

    gamma = np.asarray(ln_gamma, np.float32)
    beta = np.asarray(ln_beta, np.float32)
    trivial_affine = bool(np.all(gamma == 1.0) and np.all(beta == 0.0))

    in_maps = []
    for c in range(NCORES):
        rows = slice(c * R_, (c + 1) * R_)
        m = {
            "x_f8": x_f8,
            "x_loc": x2[rows],
            "rfT_f8": (np.ascontiguousarray(rf[rows].T) * 512.0).astype(
                ml_dtypes.float8_e4m3),
            "gT": gT_h.astype(np.float32),
            "w3T": w3T_h.astype(np.float32),
            "wp0T": wp0T, "wp1T": wp1T, "wa0T": wa0T, "wa1T": wa1T,
            "gate_col": gate_col,
        }
        if not trivial_affine:
            m["gamma_rep"] = np.broadcast_to(gamma, (128, C_)).copy()
            m["beta_rep"] = np.broadcast_to(beta, (128, C_)).copy()
        in_maps.append(m)
    return in_maps, trivial_affine


def run(x, prior, W1, W2, W3, prior_fwd_w, adaptive_w, ln_gamma, ln_beta,
        alpha, trace=False):
    in_maps, trivial_affine = prepare_in_maps(
        x, prior, W1, W2, W3, prior_fwd_w, adaptive_w, ln_gamma, ln_beta, alpha)
    nc = _get_graph(trivial_affine)
    res = run_bass_kernel_spmd(nc, in_maps, core_ids=list(range(NCORES)),
                               trace=trace)
    out = np.concatenate([np.asarray(res.results[c]["out_loc"])
                          for c in range(NCORES)], axis=0)
    return out.reshape(NP_, NC_, C_), res


def kernel(x, prior, W1, W2, W3, prior_fwd_w, adaptive_w, ln_gamma, ln_beta,
           alpha):
    out, _ = run(x, prior, W1, W2, W3, prior_fwd_w, adaptive_w, ln_gamma,
                 ln_beta, alpha, trace=False)
    return out



# revision 3
# speedup vs baseline: 1.0054x; 1.0054x over previous
"""Trainium2 Bass kernel v3 for AdaptiveDiffusionBlock (8 NeuronCores, SPMD).

All-local formulation -- no P/Q AllGathers, no E passes:
  residual = (Rf@x)@pw0T + (Rf2@x)@pw1T + (attn0@x)@aw0T + (M@x)@aw1T
where Rf2 = Rf@Rf is HOST-precomputed and M = attn1_loc@attn0_full is
computed on device after a small (4 MiB) AllGather of attn0.

Collectives: pooled0 (1 MiB), pooled1 (1 MiB), attn0 (4 MiB) -- all
overlapped with the Xf/Xa diffusion passes.

Both diffusion passes use the fused N=512 moving operand
([rfT|rf2T] resp. [attn0T|MT] per j-chunk) with x chunks stationary;
projections accumulate the two per-path terms straight into one psum
-> residual. LayerNorm is pipelined per 1024-col chunk into the Xa
pass (mi=0 Vector bn_stats, mi=1 GpSimd tensor_tensor path).
"""

import sys

for _p in ("/opt/trn_rl_repo", "/root/.axon_site", "/root/.axon_site/_ro/trn_rl_repo"):
    if _p not in sys.path:
        sys.path.append(_p)

import numpy as np
import ml_dtypes

from concourse import bacc, tile, mybir, masks
from concourse.bass_utils import run_bass_kernel_spmd

BF16 = mybir.dt.bfloat16
F32 = mybir.dt.float32
F32R = mybir.dt.float32r
F8 = mybir.dt.float8e4
AX = mybir.AxisListType
OP = mybir.AluOpType
AF = mybir.ActivationFunctionType
DR = mybir.MatmulPerfMode.DoubleRow

NCORES = 8
NP_ = 2048
NC_ = 64
C_ = 128
D_ = 64
R_ = NP_ // NCORES   # 256
KC = NC_ * C_        # 8192
P_TOPP = 0.9
LN_EPS = 1e-5
N_ITER = 5
GROUPS = [list(range(NCORES))]
SCALE_STAT = 512.0   # fp8 scale on rfT/rf2T/attn0T/MT/attn0_full
SCALE_X = 32.0       # fp8 scale on XT chunks
N_HELD = 3


def _attention_step(nc, pools, pooledT_loc, pooled_fullT, step):
    """pooledT_loc [128c,256i], pooled_fullT [128c,2048j] (f32r) ->
    two attn tiles u [128, 2048] bf16, renormalized and pre-scaled by
    SCALE_STAT."""
    psum_a, small = pools["psum_a"], pools["small"]
    big_lg, big_u = pools["big_lg"], pools["big_u"]
    gT_sb, w3T_sb = pools["gT_sb"], pools["w3T_sb"]

    qT_ps = psum_a.tile([64, R_], F32, name=f"qT_ps{step}", tag="attn_ps")
    nc.tensor.matmul(qT_ps[:], lhsT=gT_sb[:], rhs=pooledT_loc,
                     start=True, stop=True)
    qT_sb = big_lg.tile([64, R_], F32R, name=f"qT_sb{step}", tag="qT_sb")
    nc.vector.tensor_copy(qT_sb[:], qT_ps[:])

    e3T_sb = big_lg.tile([64, NP_], F32R, name=f"e3T_sb{step}", tag="e3T_sb")
    for n in range(4):
        e3_ps = psum_a.tile([64, 512], F32, name=f"e3_ps{step}_{n}", tag="attn_ps")
        nc.tensor.matmul(e3_ps[:], lhsT=w3T_sb[:],
                         rhs=pooled_fullT[:, n * 512:(n + 1) * 512],
                         start=True, stop=True)
        nc.vector.tensor_copy(e3T_sb[:, n * 512:(n + 1) * 512], e3_ps[:])

    attn_tiles = []
    for mi in range(2):
        lg = big_lg.tile([128, NP_], F32, name=f"lg{step}_{mi}", tag="logits")
        for n in range(4):
            lg_ps = psum_a.tile([128, 512], F32, name=f"lg_ps{step}_{mi}_{n}",
                                tag="attn_ps")
            nc.tensor.matmul(lg_ps[:],
                             lhsT=qT_sb[:, mi * 128:(mi + 1) * 128],
                             rhs=e3T_sb[:, n * 512:(n + 1) * 512],
                             start=True, stop=True)
            nc.vector.tensor_copy(lg[:, n * 512:(n + 1) * 512], lg_ps[:])

        rmax = small.tile([128, 1], F32, name=f"rmax{step}_{mi}", tag="rmax")
        nc.vector.tensor_reduce(rmax[:], lg[:], axis=AX.X, op=OP.max)
        negmax = small.tile([128, 1], F32, name=f"negmax{step}_{mi}", tag="negmax")
        nc.vector.tensor_scalar(negmax[:], rmax[:], -1.0, None, op0=OP.mult)
        u = big_u.tile([128, NP_], BF16, name=f"u{step}_{mi}", tag="u")
        zp = small.tile([128, 4], F32, name=f"zp{step}_{mi}", tag="zp")
        for n in range(4):
            nc.scalar.activation(u[:, n * 512:(n + 1) * 512],
                                 lg[:, n * 512:(n + 1) * 512],
                                 AF.Exp, bias=negmax[:], scale=1.0,
                                 accum_out=zp[:, n:n + 1])
        target = small.tile([128, 1], F32, name=f"target{step}_{mi}", tag="target")
        nc.vector.tensor_reduce(target[:], zp[:], axis=AX.X, op=OP.add)
        nc.vector.tensor_scalar(target[:], target[:], P_TOPP, None, op0=OP.mult)

        t = small.tile([128, 1], F32, name=f"t{step}_{mi}", tag="t")
        t_lo = small.tile([128, 1], F32, name=f"tlo{step}_{mi}", tag="tlo")
        nc.vector.memset(t[:], 0.5)
        nc.vector.memset(t_lo[:], 0.0)
        scratch = big_u.tile([128, NP_], BF16, name=f"scr{step}_{mi}", tag="scratch")
        hsum = small.tile([128, 1], F32, name=f"h{step}_{mi}", tag="hsum")
        cond = small.tile([128, 1], F32, name=f"cond{step}_{mi}", tag="cond")
        tdif = small.tile([128, 1], F32, name=f"tdif{step}_{mi}", tag="tdif")
        toff = small.tile([128, 1], F32, name=f"toff{step}_{mi}", tag="toff")
        for it in range(N_ITER):
            nc.vector.scalar_tensor_tensor(scratch[:], u[:], t[:], u[:],
                                           op0=OP.is_gt, op1=OP.mult,
                                           accum_out=hsum[:])
            nc.vector.tensor_tensor(cond[:], hsum[:], target[:], op=OP.is_ge)
            nc.vector.tensor_tensor(tdif[:], t[:], t_lo[:], op=OP.subtract)
            nc.vector.scalar_tensor_tensor(t_lo[:], tdif[:], cond[:], t_lo[:],
                                           op0=OP.mult, op1=OP.add)
            delta = 2.0 ** (-(it + 2))
            nc.vector.tensor_scalar(toff[:], t[:], -delta, None, op0=OP.add)
            nc.vector.scalar_tensor_tensor(t[:], cond[:], 2.0 * delta, toff[:],
                                           op0=OP.mult, op1=OP.add)

        ssum = small.tile([128, 1], F32, name=f"S{step}_{mi}", tag="ssum")
        nc.vector.scalar_tensor_tensor(scratch[:], u[:], t_lo[:], u[:],
                                       op0=OP.is_gt, op1=OP.mult,
                                       accum_out=ssum[:])
        rs = small.tile([128, 1], F32, name=f"rS{step}_{mi}", tag="rs")
        nc.vector.reciprocal(rs[:], ssum[:])
        nc.vector.tensor_scalar(rs[:], rs[:], SCALE_STAT, None, op0=OP.mult)
        nc.vector.tensor_scalar(u[:], scratch[:], rs[:], None, op0=OP.mult)
        attn_tiles.append(u)
    return attn_tiles


def build_graph(trivial_affine=True):
    nc = bacc.Bacc("TRN2", target_bir_lowering=False, debug=False,
                   num_devices=NCORES)

    x_f8 = nc.dram_tensor("x_f8", [NP_, KC], F8, kind="ExternalInput")
    x_loc = nc.dram_tensor("x_loc", [R_, KC], F32, kind="ExternalInput")
    # catF rows jb*128..+128: [rfT chunk (256 i) | rf2T chunk (256 i)]
    catF = nc.dram_tensor("catF", [NP_, 512], F8, kind="ExternalInput")
    gT = nc.dram_tensor("gT", [C_, D_], F32, kind="ExternalInput")
    w3T = nc.dram_tensor("w3T", [C_, D_], F32, kind="ExternalInput")
    wcat2_f = nc.dram_tensor("wcat2_f", [2 * C_, C_], F8, kind="ExternalInput")
    wcat2_a = nc.dram_tensor("wcat2_a", [2 * C_, C_], F8, kind="ExternalInput")
    gate_col = nc.dram_tensor("gate_col", [128, 1], F32, kind="ExternalInput")
    if not trivial_affine:
        gamma_rep = nc.dram_tensor("gamma_rep", [128, C_], F32,
                                   kind="ExternalInput")
        beta_rep = nc.dram_tensor("beta_rep", [128, C_], F32,
                                  kind="ExternalInput")
    out_loc = nc.dram_tensor("out_loc", [R_, KC], F32, kind="ExternalOutput")

    with tile.TileContext(nc) as tc:
        dram = tc.alloc_tile_pool(name="dram", bufs=1, space="DRAM")
        pooled0_bounce = dram.tile([R_, C_], F32, name="pooled0_bounce")
        pooled0_full = dram.tile([NP_, C_], F32, name="pooled0_full",
                                 addr_space="Shared")
        pooled1_bounce = dram.tile([R_, C_], F32, name="pooled1_bounce")
        pooled1_full = dram.tile([NP_, C_], F32, name="pooled1_full",
                                 addr_space="Shared")
        att0_bounce = dram.tile([R_, NP_], F8, name="att0_bounce")
        att0_full = dram.tile([NP_, NP_], F8, name="att0_full",
                              addr_space="Shared")

        const_pool = tc.alloc_tile_pool(name="const", bufs=1)
        small_pool = tc.alloc_tile_pool(name="small", bufs=2)

        ident_f32 = const_pool.tile([128, 128], F32, name="ident_f32")
        ident_bf16 = const_pool.tile([128, 128], BF16, name="ident_bf16")
        ident_f8 = const_pool.tile([128, 128], F8, name="ident_f8")
        masks.make_identity(nc, ident_f32[:])
        masks.make_identity(nc, ident_bf16[:])
        nc.vector.tensor_copy(ident_f8[:], ident_bf16[:])

        gT_f32 = const_pool.tile([C_, D_], F32, name="gT_f32")
        w3T_f32 = const_pool.tile([C_, D_], F32, name="w3T_f32")
        nc.scalar.dma_start(gT_f32[:], gT[:, :])
        nc.scalar.dma_start(w3T_f32[:], w3T[:, :])
        gT_sb = const_pool.tile([C_, D_], F32R, name="gT_sb")
        w3T_sb = const_pool.tile([C_, D_], F32R, name="w3T_sb")
        nc.vector.tensor_copy(gT_sb[:], gT_f32[:])
        nc.vector.tensor_copy(w3T_sb[:], w3T_f32[:])
        wps = {}
        for nm, hd in (("wf", wcat2_f), ("wa", wcat2_a)):
            wsb = const_pool.tile([128, 2, C_], F8, name=nm + "_sb")
            nc.scalar.dma_start(wsb[:],
                                hd[:, :].rearrange("(s p) d -> p s d", p=128))
            wps[nm] = wsb
        gate_sb = const_pool.tile([128, 1], F32, name="gate_sb")
        nc.scalar.dma_start(gate_sb[:], gate_col[:, :])
        if not trivial_affine:
            gamma_sb = const_pool.tile([128, C_], F32, name="gamma_sb")
            beta_sb = const_pool.tile([128, C_], F32, name="beta_sb")
            nc.scalar.dma_start(gamma_sb[:], gamma_rep[:, :])
            nc.scalar.dma_start(beta_sb[:], beta_rep[:, :])

        # fused moving operands: catF (rfT|rf2T, from host) and catA
        # (attn0T|MT, built on device)
        catF_sb = const_pool.tile([128, 16 * 512], F8, name="catF_sb")
        nc.sync.dma_start(
            catF_sb[:].rearrange("p (jb i) -> p jb i", jb=16),
            catF[:, :].rearrange("(jb p) i -> p jb i", p=128))
        catA_sb = const_pool.tile([128, 16 * 512], F8, name="catA_sb")
        attn1T_sb = const_pool.tile([128, 16 * 256], F8, name="attn1T_sb")
        catF_v = catF_sb[:].rearrange("p (jb s i) -> p jb s i", s=2, i=512)
        catA_v = catA_sb[:].rearrange("p (jb s i) -> p jb s i", s=2, i=512)
        a1_v = attn1T_sb[:].rearrange("p (jb s i) -> p jb s i", s=2, i=256)

        resid = [[const_pool.tile([128, 2048], BF16, name=f"resid{mi}_{cc}")
                  for cc in range(4)] for mi in range(2)]
        _pfT = const_pool.tile([128, NP_], F32R, name="pooled_fullT")
        pooled0T_loc = const_pool.tile([128, R_], F32R, name="pooled0T_loc")
        pooled1T_loc = const_pool.tile([128, R_], F32R, name="pooled1T_loc")
        p08_sb = const_pool.tile([128, NP_], F8, name="p08_sb")

        psum_b = tc.alloc_tile_pool(name="psum_b", bufs=2, space="PSUM")
        psum_c = tc.alloc_tile_pool(name="psum_c", bufs=2, space="PSUM")
        psum_a = tc.alloc_tile_pool(name="psum_a", bufs=2, space="PSUM")
        psum_tp = tc.alloc_tile_pool(name="psum_tp", bufs=2, space="PSUM")

        big_lg = tc.alloc_tile_pool(name="big_lg", bufs=1)
        big_u = tc.alloc_tile_pool(name="big_u", bufs=2)
        pools = dict(psum_a=psum_a, small=small_pool, big_lg=big_lg,
                     big_u=big_u, gT_sb=gT_sb, w3T_sb=w3T_sb)

        xheld_pool = tc.alloc_tile_pool(name="xheld", bufs=1)
        xs_pool = tc.alloc_tile_pool(name="xstream", bufs=2)
        ln_pool = tc.alloc_tile_pool(name="ln_pool", bufs=2)
        xfxa_pool = tc.alloc_tile_pool(name="xfxa", bufs=3)
        xt_held = {}

        def load_x_tiles(kq, held):
            xts = []
            for jh in range(2):
                if held:
                    xt = xheld_pool.tile([128, 8, 512], F8, name=f"xh{kq}_{jh}")
                else:
                    xt = xs_pool.tile([128, 8, 512], F8,
                                      name=f"xs{kq}_{jh}", tag=f"xs{jh}")
                src_ = x_f8[jh * 1024:(jh + 1) * 1024,
                            kq * 512:(kq + 1) * 512].rearrange(
                                "(jb p) c -> p jb c", p=128)
                nc.sync.dma_start(xt[:, :4, :], src_[:, :4, :])
                nc.sync.dma_start(xt[:, 4:, :], src_[:, 4:, :])
                xts.append(xt)
            return xts

        KW = 8  # k-groups per 1024-col LN chunk

        def layernorm_chunk(n2, mi):
            """LN for cols [n2*1024, (n2+1)*1024), rows mi*128..+128."""
            cs = slice(n2 * 1024, (n2 + 1) * 1024)
            cc = n2 // 2
            rsl = resid[mi][cc][:, (n2 % 2) * 1024:(n2 % 2 + 1) * 1024]
            xl2 = ln_pool.tile([128, 1024], F32, name=f"xl2_{mi}_{n2}",
                               tag=f"xl2_{mi}")
            nc.sync.dma_start(xl2[:], x_loc[mi * 128:(mi + 1) * 128, cs])
            h = ln_pool.tile([128, 1024], BF16, name=f"hln_{mi}_{n2}",
                             tag=f"hln_{mi}")
            if mi == 0:
                nc.vector.scalar_tensor_tensor(h[:], rsl, gate_sb[:], xl2[:],
                                               op0=OP.mult, op1=OP.add)
            else:
                gate_bc = gate_sb[:].broadcast_to([128, 1024])
                nc.gpsimd.tensor_tensor(h[:], rsl, gate_bc, op=OP.mult)
                nc.gpsimd.tensor_tensor(h[:], h[:], xl2[:], op=OP.add)
            hv = h[:].rearrange("p (k c) -> p k c", k=KW)
            rstd = ln_pool.tile([128, KW, 1], F32, name=f"rstd_{mi}_{n2}",
                                tag=f"rstd_{mi}")
            mb = ln_pool.tile([128, KW, 1], F32, name=f"mb_{mi}_{n2}",
                              tag=f"mb_{mi}")
            var = ln_pool.tile([128, KW, 1], F32, name=f"var_{mi}_{n2}",
                               tag=f"var_{mi}")
            sd = ln_pool.tile([128, KW, 1], F32, name=f"sd_{mi}_{n2}",
                              tag=f"sd_{mi}")
            st6 = ln_pool.tile([128, KW, 6], F32, name=f"st6_{mi}_{n2}",
                               tag=f"st6_{mi}")
            mv = ln_pool.tile([128, KW, 2], F32, name=f"mv_{mi}_{n2}",
                              tag=f"mv_{mi}")
            for g in range(KW):
                nc.vector.bn_stats(st6[:, g, :], hv[:, g, :])
                nc.vector.bn_aggr(mv[:, g, :], st6[:, g, :])
            nc.vector.tensor_scalar(var[:], mv[:, :, 1:2], 1.0, LN_EPS,
                                    op0=OP.mult, op1=OP.add)
            nc.scalar.activation(sd[:], var[:], AF.Sqrt)
            nc.vector.reciprocal(rstd[:], sd[:])
            nc.vector.tensor_tensor(mb[:], mv[:, :, 0:1], rstd[:], op=OP.mult)
            nc.vector.tensor_scalar(mb[:], mb[:], -1.0, None, op0=OP.mult)
            ov = xl2[:].rearrange("p (k c) -> p k c", k=KW)
            if mi == 0:
                for g in range(KW):
                    mb_bc = mb[:, g, :].broadcast_to([128, C_])
                    nc.vector.scalar_tensor_tensor(ov[:, g, :], hv[:, g, :],
                                                   rstd[:, g, :], mb_bc,
                                                   op0=OP.mult, op1=OP.add)
            else:
                rstd_bc = rstd[:].broadcast_to([128, KW, C_])
                mb_bc = mb[:].broadcast_to([128, KW, C_])
                nc.gpsimd.tensor_tensor(ov, hv, rstd_bc, op=OP.mult)
                nc.gpsimd.tensor_tensor(ov, ov, mb_bc, op=OP.add)
            if not trivial_affine:
                eng = nc.vector if mi == 0 else nc.gpsimd
                g_bc = gamma_sb[:].rearrange(
                    "p (one c) -> p one c", one=1).broadcast_to([128, KW, C_])
                b_bc = beta_sb[:].rearrange(
                    "p (one c) -> p one c", one=1).broadcast_to([128, KW, C_])
                eng.tensor_tensor(ov, ov, g_bc, op=OP.mult)
                eng.tensor_tensor(ov, ov, b_bc, op=OP.add)
            nc.scalar.dma_start(out_loc[mi * 128:(mi + 1) * 128, cs], xl2[:])

        def stage_b_kq(kq, xts, mov_v, wcat2_sb, pass_id):
            """One kq group: 4 k-slices of XT|X2T accumulation plus the
            fused two-term projection straight into the residual."""
            use_scalar = pass_id == 0 and kq >= 11
            xT = xfxa_pool.tile([128, 4, 512], F8, name=f"xT{pass_id}_{kq}",
                                tag="xT")
            for k4 in range(4):
                ps = psum_b.tile([128, 512], F32, name=f"psB{pass_id}_{kq}_{k4}",
                                 tag="psB")
                for jbp in range(8):
                    nc.tensor.matmul(
                        ps[:],
                        lhsT=xts[jbp // 4][:, (jbp % 4) * 2:(jbp % 4) * 2 + 2,
                                           k4 * 128:(k4 + 1) * 128],
                        rhs=mov_v[:, jbp, :, :],
                        start=(jbp == 0), stop=(jbp == 7),
                        perf_mode=DR)
                nc.scalar.mul(xT[:, k4, :], ps[:], SCALE_X / SCALE_STAT)
            xT_v = xT[:].rearrange("p k (s i) -> p k s i", s=2)
            for ig in range(2):
                psp = psum_c.tile([128, 512], F32,
                                  name=f"psP{pass_id}_{kq}_{ig}", tag="psP")
                for k4 in range(4):
                    nc.tensor.matmul(psp[:, k4 * 128:(k4 + 1) * 128],
                                     lhsT=xT_v[:, k4, :,
                                               ig * 128:(ig + 1) * 128],
                                     rhs=wcat2_sb[:], start=True, stop=True,
                                     perf_mode=DR)
                rsl = resid[ig][kq // 4][:, (kq % 4) * 512:(kq % 4 + 1) * 512]
                if pass_id == 0:
                    if use_scalar:
                        nc.scalar.mul(rsl, psp[:], 1.0 / 256.0)
                    else:
                        nc.vector.tensor_scalar(rsl, psp[:], 1.0 / 256.0,
                                                None, op0=OP.mult)
                else:
                    nc.vector.scalar_tensor_tensor(rsl, psp[:], 1.0 / 256.0,
                                                   rsl, op0=OP.mult,
                                                   op1=OP.add)

        # ================= pooled0 (chunked pipeline) =================
        pooled0_rows = [small_pool.tile([128, C_], F32, name=f"pooled0_r{mi}",
                                        tag=f"pooled0_r{mi}") for mi in range(2)]
        pacc = [small_pool.tile([128, C_], F32, name=f"pacc{mi}",
                                tag=f"pacc{mi}") for mi in range(2)]
        with tc.tile_pool(name="xl_pool", bufs=2) as xl_pool:
            for mi in range(2):
                for ch in range(16):
                    xl = xl_pool.tile([128, 512], F32, name=f"xl{mi}_{ch}",
                                      tag=f"xl{mi}")
                    nc.scalar.dma_start(
                        xl[:], x_loc[mi * 128:(mi + 1) * 128,
                                     ch * 512:(ch + 1) * 512])
                    v = xl[:].rearrange("p (k c) -> p c k", k=4)
                    if ch == 0:
                        nc.vector.tensor_reduce(pacc[mi][:], v, axis=AX.X,
                                                op=OP.add)
                    else:
                        part = xl_pool.tile([128, C_], F32,
                                            name=f"pp{mi}_{ch}", tag="pp")
                        nc.vector.tensor_reduce(part[:], v, axis=AX.X,
                                                op=OP.add)
                        dst = (pooled0_rows[mi][:] if ch == 15
                               else pacc[mi][:])
                        nc.vector.tensor_tensor(dst, pacc[mi][:], part[:],
                                                op=OP.add)
                nc.gpsimd.dma_start(pooled0_bounce[mi * 128:(mi + 1) * 128, :],
                                    pooled0_rows[mi][:])
                ps = psum_tp.tile([128, 128], F32, name=f"tp_p0_{mi}", tag="tp")
                nc.tensor.transpose(ps[:], pooled0_rows[mi][:], ident_f32[:])
                nc.vector.tensor_copy(pooled0T_loc[:, mi * 128:(mi + 1) * 128],
                                      ps[:])
        nc.gpsimd.collective_compute(
            "AllGather", OP.bypass, replica_groups=GROUPS,
            ins=[pooled0_bounce[:, :]], outs=[pooled0_full[:, :]])

        # ================= Xf pass (Rf | Rf2) =================
        for kq in range(16):
            held = kq < N_HELD
            xts = load_x_tiles(kq, held)
            if held:
                xt_held[kq] = xts
            stage_b_kq(kq, xts, catF_v, wps["wf"], 0)
            if kq == 11:
                with tc.tile_pool(name="pf_pool", bufs=3) as pf_pool:
                    for jq in range(4):
                        pt_ = pf_pool.tile([128, 4, C_], F32, name="pf_t",
                                           tag="pf_t")
                        src_ = pooled0_full[jq * 512:(jq + 1) * 512, :].rearrange(
                            "(jb p) c -> p jb c", p=128)
                        nc.gpsimd.dma_start(pt_[:], src_)
                        for j4 in range(4):
                            jb = jq * 4 + j4
                            psx = psum_tp.tile([128, 128], F32,
                                               name=f"tp_pf0_{jb}", tag="tp")
                            nc.tensor.transpose(psx[:], pt_[:, j4, :],
                                                ident_f32[:])
                            nc.vector.tensor_copy(
                                _pfT[:, jb * 128:(jb + 1) * 128], psx[:])
                            nc.scalar.copy(
                                p08_sb[:, jb * 128:(jb + 1) * 128],
                                pt_[:, j4, :])
                attn0 = _attention_step(nc, pools, pooled0T_loc[:], _pfT[:], 0)

        # attn0 -> transposed chunks into catA (first 256-col half of
        # each jb slot) + row-major fp8 staging for the attn0 AllGather
        for mi in range(2):
            for jb in range(16):
                ps = psum_tp.tile([128, 128], BF16, name=f"tpa0_{mi}_{jb}",
                                  tag="tp")
                nc.tensor.transpose(ps[:], attn0[mi][:, jb * 128:(jb + 1) * 128],
                                    ident_bf16[:])
                nc.scalar.copy(catA_sb[:, jb * 512 + mi * 128:
                                       jb * 512 + (mi + 1) * 128], ps[:])
        for mi in range(2):
            arow = ln_pool.tile([128, NP_], F8, name=f"att0row{mi}",
                                tag="att0row")
            nc.vector.tensor_copy(arow[:], attn0[mi][:])
            nc.gpsimd.dma_start(att0_bounce[mi * 128:(mi + 1) * 128, :],
                                arow[:])

        # pooled1 = attn0 @ pooled0 (T-layout)
        ps1 = psum_a.tile([128, R_], F32, name="pooled1_ps", tag="attn_ps")
        for jb in range(16):
            nc.tensor.matmul(ps1[:],
                             lhsT=p08_sb[:, jb * 128:(jb + 1) * 128],
                             rhs=catA_sb[:, jb * 512:jb * 512 + 256],
                             start=(jb == 0), stop=(jb == 15))
        nc.vector.tensor_scalar(pooled1T_loc[:], ps1[:], 1.0 / SCALE_STAT,
                                None, op0=OP.mult)
        pooled1_rows = [small_pool.tile([128, C_], F32, name=f"pooled1_r{mi}",
                                        tag=f"pooled1_r{mi}") for mi in range(2)]
        ident_f32r = const_pool.tile([128, 128], F32R, name="ident_f32r")
        nc.vector.tensor_copy(ident_f32r[:], ident_f32[:])
        for mi in range(2):
            ps = psum_tp.tile([128, 128], F32R, name=f"tp_p1_{mi}", tag="tp")
            nc.tensor.transpose(ps[:],
                                pooled1T_loc[:, mi * 128:(mi + 1) * 128],
                                ident_f32r[:])
            nc.vector.tensor_copy(pooled1_rows[mi][:], ps[:])
            nc.gpsimd.dma_start(pooled1_bounce[mi * 128:(mi + 1) * 128, :],
                                pooled1_rows[mi][:])

        nc.gpsimd.collective_compute(
            "AllGather", OP.bypass, replica_groups=GROUPS,
            ins=[pooled1_bounce[:, :]], outs=[pooled1_full[:, :]])
        nc.gpsimd.collective_compute(
            "AllGather", OP.bypass, replica_groups=GROUPS,
            ins=[att0_bounce[:, :]], outs=[att0_full[:, :]])

        # ================= attention1 =================
        with tc.tile_pool(name="pf1_pool", bufs=3) as pf1_pool:
            for jq in range(4):
                pt_ = pf1_pool.tile([128, 4, C_], F32, name="pf1_t",
                                    tag="pf1_t")
                src_ = pooled1_full[jq * 512:(jq + 1) * 512, :].rearrange(
                    "(jb p) c -> p jb c", p=128)
                nc.gpsimd.dma_start(pt_[:], src_)
                for j4 in range(4):
                    jb = jq * 4 + j4
                    psx = psum_tp.tile([128, 128], F32,
                                       name=f"tp_pf1_{jb}", tag="tp")
                    nc.tensor.transpose(psx[:], pt_[:, j4, :], ident_f32[:])
                    nc.vector.tensor_copy(_pfT[:, jb * 128:(jb + 1) * 128],
                                          psx[:])
        attn1 = _attention_step(nc, pools, pooled1T_loc[:], _pfT[:], 1)
        for mi in range(2):
            for jb in range(16):
                ps = psum_tp.tile([128, 128], BF16, name=f"tpa1_{mi}_{jb}",
                                  tag="tp")
                nc.tensor.transpose(ps[:], attn1[mi][:, jb * 128:(jb + 1) * 128],
                                    ident_bf16[:])
                nc.scalar.copy(attn1T_sb[:, jb * 256 + mi * 128:
                                         jb * 256 + (mi + 1) * 128], ps[:])

        # ================= M = attn1_loc @ attn0_full =================
        m_sb = [ln_pool.tile([128, NP_], BF16, name=f"m_sb{mi}",
                             tag=f"m_sb{mi}") for mi in range(2)]
        with tc.tile_pool(name="m_rhs", bufs=1) as m_pool:
            for n in range(4):
                srcv = att0_full[:, n * 512:(n + 1) * 512].rearrange(
                    "(jb p) j -> p jb j", p=128)
                rts = []
                for hh in range(2):
                    rt = m_pool.tile([128, 8, 512], F8, name=f"mrt{n}_{hh}",
                                     tag=f"mrt{hh}")
                    nc.sync.dma_start(rt[:], srcv[:, hh * 8:(hh + 1) * 8, :])
                    rts.append(rt)
                for mi in range(2):
                    psm = psum_b.tile([128, 512], F32, name=f"psM{n}_{mi}",
                                      tag="psB")
                    for jb in range(8):
                        nc.tensor.matmul(
                            psm[:],
                            lhsT=a1_v[:, jb, :, mi * 128:(mi + 1) * 128],
                            rhs=rts[jb // 4][:, (jb % 4) * 2:(jb % 4) * 2 + 2, :],
                            start=(jb == 0), stop=(jb == 7),
                            perf_mode=DR)
                    nc.scalar.mul(m_sb[mi][:, n * 512:(n + 1) * 512], psm[:],
                                  1.0 / SCALE_STAT)
        # MT transposes into catA second halves
        for mi in range(2):
            for jb in range(16):
                ps = psum_tp.tile([128, 128], BF16, name=f"tpm_{mi}_{jb}",
                                  tag="tp")
                nc.tensor.transpose(ps[:], m_sb[mi][:, jb * 128:(jb + 1) * 128],
                                    ident_bf16[:])
                nc.scalar.copy(catA_sb[:, jb * 512 + 256 + mi * 128:
                                       jb * 512 + 256 + (mi + 1) * 128], ps[:])

        # ================= Xa pass (attn0 | M) + pipelined LN ==========
        for kq in range(16):
            xts = xt_held.get(kq)
            if xts is None:
                xts = load_x_tiles(kq, False)
            stage_b_kq(kq, xts, catA_v, wps["wa"], 1)
            if kq % 2 == 1:
                layernorm_chunk(kq // 2, 0)
                layernorm_chunk(kq // 2, 1)

        xfxa_pool.release()
        ln_pool.release()
        xs_pool.release()
        xheld_pool.release()
        big_u.release()
        big_lg.release()
        psum_tp.release()
        psum_a.release()
        psum_c.release()
        psum_b.release()
        small_pool.release()
        const_pool.release()
        dram.release()

    nc.finalize()
    return nc


# ---------------------------------------------------------------------------
# Host side
# ---------------------------------------------------------------------------
_CACHE = {}


def _get_graph(trivial_affine):
    key = bool(trivial_affine)
    if key not in _CACHE:
        _CACHE[key] = build_graph(key)
    return _CACHE[key]


def prepare_in_maps(x, prior, W1, W2, W3, prior_fwd_w, adaptive_w,
                    ln_gamma, ln_beta, alpha):
    f8 = ml_dtypes.float8_e4m3
    x2 = np.ascontiguousarray(np.asarray(x, np.float32).reshape(NP_, KC))
    x_f8 = x2.astype(f8)
    prior = np.asarray(prior, np.float32)
    rs = np.maximum(prior.sum(axis=1, keepdims=True), 1e-12)
    rf = (prior / rs).astype(np.float32)
    rf2 = (rf @ rf).astype(np.float32)

    W1 = np.asarray(W1, np.float32)
    W2 = np.asarray(W2, np.float32)
    W3 = np.asarray(W3, np.float32)
    G = (W2 @ W1)
    gT_h = np.ascontiguousarray(G.T) / np.float32(NC_)
    w3T_h = np.ascontiguousarray(W3.T) / np.float32(NC_)

    pw = np.asarray(prior_fwd_w, np.float32)
    aw = np.asarray(adaptive_w, np.float32)
    wcat2_f_h = (np.concatenate([pw[0].T, pw[1].T], axis=0)
                 * 8.0).astype(f8)
    wcat2_a_h = (np.concatenate([aw[0].T, aw[1].T], axis=0)
                 * 8.0).astype(f8)

    gate = 1.0 / (1.0 + np.exp(-np.float32(np.asarray(alpha).reshape(-1)[0])))
    gate_col = np.full((128, 1), gate, np.float32)

    gamma = np.asarray(ln_gamma, np.float32)
    beta = np.asarray(ln_beta, np.float32)
    trivial_affine = bool(np.all(gamma == 1.0) and np.all(beta == 0.0))

    in_maps = []
    for c in range(NCORES):
        rows = slice(c * R_, (c + 1) * R_)
        rfT = np.ascontiguousarray(rf[rows].T) * SCALE_STAT    # [2048, 256]
        rf2T = np.ascontiguousarray(rf2[rows].T) * SCALE_STAT
        catF_h = np.concatenate([rfT, rf2T], axis=1).astype(f8)  # [2048, 512]
        m = {
            "x_f8": x_f8,
            "x_loc": x2[rows],
            "catF": catF_h,
            "gT": gT_h.astype(np.float32),
            "w3T": w3T_h.astype(np.float32),
            "wcat2_f": wcat2_f_h, "wcat2_a": wcat2_a_h,
            "gate_col": gate_col,
        }
        if not trivial_affine:
            m["gamma_rep"] = np.broadcast_to(gamma, (128, C_)).copy()
            m["beta_rep"] = np.broadcast_to(beta, (128, C_)).copy()
        in_maps.append(m)
    return in_maps, trivial_affine


def run(x, prior, W1, W2, W3, prior_fwd_w, adaptive_w, ln_gamma, ln_beta,
        alpha, trace=False):
    in_maps, trivial_affine = prepare_in_maps(
        x, prior, W1, W2, W3, prior_fwd_w, adaptive_w, ln_gamma, ln_beta, alpha)
    nc = _get_graph(trivial_affine)
    res = run_bass_kernel_spmd(nc, in_maps, core_ids=list(range(NCORES)),
                               trace=trace)
    out = np.concatenate([np.asarray(res.results[c]["out_loc"])
                          for c in range(NCORES)], axis=0)
    return out.reshape(NP_, NC_, C_), res


def kernel(x, prior, W1, W2, W3, prior_fwd_w, adaptive_w, ln_gamma, ln_beta,
           alpha):
    out, _ = run(x, prior, W1, W2, W3, prior_fwd_w, adaptive_w, ln_gamma,
                 ln_beta, alpha, trace=False)
    return out
